# revision 1
# baseline (speedup 1.0000x reference)
"""Trainium2 Bass kernel for nn_LGL GNN message passing (N=64, K=32, F=1024).

Data-parallel over nodes: 8 nodes per core on 8 NeuronCores. Layer-1
adjacency uses sign(fadj) (exact to ~1e-6: the row-normalization for
c=1 reduces to r/(r+1e-7) with r >= 1e-4, i.e. sign() up to <=1e-3 on a
measure-zero set). BN1 x-stats and layer 2 need cross-node info: the
kernel AllGathers pre-BN x1 plus S2 = sum_k softsign(BN(nb1)) (64x16
floats per core) and every core redundantly computes the tiny layer 2
for all 64 nodes.
"""
import numpy as np

N_CORES = 8
NPC = 8          # nodes per core
F = 1024
K = 32
BN_EPS = 1e-5

_CACHE = {}
_DEBUG = False
_SKIP_SIGN = False
_SKIP_NA = False
_SKIP_FADJ = False
_SKIP_TPOSE = False


def _build():
    global _SKIP_SIGN, _SKIP_NA, _SKIP_FADJ, _SKIP_TPOSE
    import concourse.bacc as bacc
    import concourse.mybir as mybir
    import concourse.tile as tile

    dt = mybir.dt.float32
    dtb = mybir.dt.bfloat16
    AX = mybir.AxisListType
    OP = mybir.AluOpType

    nc = bacc.Bacc("TRN2", target_bir_lowering=False, debug=False)

    xs = nc.dram_tensor("xs", [NPC, F], dt, kind="ExternalInput")
    nbs = nc.dram_tensor("nbs", [NPC, K, F], dt, kind="ExternalInput")
    w1t = nc.dram_tensor("w1t", [8, 128, 64], dt, kind="ExternalInput")
    sel = nc.dram_tensor("sel", [33, 2], dt, kind="ExternalInput")
    idt = nc.dram_tensor("idt", [128, 128], dt, kind="ExternalInput")
    g4 = nc.dram_tensor("g4", [64, 4], dt, kind="ExternalInput")
    bc4 = nc.dram_tensor("bc4", [4, 64], dt, kind="ExternalInput")
    bnw1 = nc.dram_tensor("bnw1", [64, 1], dt, kind="ExternalInput")
    bnb1 = nc.dram_tensor("bnb1", [64, 1], dt, kind="ExternalInput")
    w2t = nc.dram_tensor("w2t", [64, 32], dt, kind="ExternalInput")
    bnw2 = nc.dram_tensor("bnw2", [32, 1], dt, kind="ExternalInput")
    bnb2 = nc.dram_tensor("bnb2", [32, 1], dt, kind="ExternalInput")
    linw = nc.dram_tensor("linw", [33, 10], dt, kind="ExternalInput")
    out_d = nc.dram_tensor("out", [64, 10], dt, kind="ExternalOutput")
    gshared = nc.dram_tensor("gshared", [N_CORES, 64, 16], dt,
                             addr_space="Shared")

    with tile.TileContext(nc) as tc:
        with (
            tc.tile_pool(name="wpool", bufs=1) as wp,
            tc.tile_pool(name="upool", bufs=2) as up,
            tc.tile_pool(name="vpool", bufs=2) as vp,
            tc.tile_pool(name="xpool", bufs=2) as xp,
            tc.tile_pool(name="rpool", bufs=10) as rp,
            tc.tile_pool(name="ypool", bufs=2) as yp,
            tc.tile_pool(name="work", bufs=1) as wk,
            tc.tile_pool(name="pfadj", bufs=3, space="PSUM") as pf,
            tc.tile_pool(name="pacc", bufs=2, space="PSUM") as pa,
            tc.tile_pool(name="pmisc", bufs=3, space="PSUM") as pm,
            tc.tile_pool(name="dram", bufs=1, space="DRAM") as dp,
        ):
            # ---- load weights / constants ----
            w1t_s = wp.tile([128, 8, 64], dt)
            nc.sync.dma_start(out=w1t_s[:], in_=w1t.ap().rearrange("c p o -> p c o"))
            sel_s = wp.tile([33, 2], dt)
            nc.sync.dma_start(out=sel_s[:], in_=sel[:])
            idt_s = wp.tile([128, 128], dt)
            nc.sync.dma_start(out=idt_s[:], in_=idt[:])
            g4_s = wp.tile([64, 4], dt)
            nc.sync.dma_start(out=g4_s[:], in_=g4[:])
            bc4_s = wp.tile([4, 64], dt)
            nc.sync.dma_start(out=bc4_s[:], in_=bc4[:])
            bnw1_s = wp.tile([64, 1], dt)
            nc.sync.dma_start(out=bnw1_s[:], in_=bnw1[:])
            bnb1_s = wp.tile([64, 1], dt)
            nc.sync.dma_start(out=bnb1_s[:], in_=bnb1[:])
            w2t_s = wp.tile([64, 32], dt)
            nc.sync.dma_start(out=w2t_s[:], in_=w2t[:])
            bnw2_s = wp.tile([32, 1], dt)
            nc.sync.dma_start(out=bnw2_s[:], in_=bnw2[:])
            bnb2_s = wp.tile([32, 1], dt)
            nc.sync.dma_start(out=bnb2_s[:], in_=bnb2[:])
            linw_s = wp.tile([33, 10], dt)
            nc.sync.dma_start(out=linw_s[:], in_=linw[:])

            # Z[o, n, j]: layer-1 raw outputs per node; j=0 x-path, 1..32 nb
            z_t = wk.tile([64, NPC, 33], dt, tag="z")

            # ======== layer 1, per local node ========
            for n in range(NPC):
                u33 = up.tile([33, F], dt, tag="u33")
                nc.sync.dma_start(out=u33[0:1, :], in_=xs[n:n + 1, :])
                nc.sync.dma_start(out=u33[1:33, :], in_=nbs[n, :, :])

                # US = [x; s] (2, F) via selector matmul
                ps_us = pm.tile([2, 512], dt, tag="m")
                ps_us2 = pm.tile([2, 512], dt, tag="m")
                nc.tensor.matmul(ps_us[:], sel_s[:], u33[:, 0:512],
                                 start=True, stop=True)
                nc.tensor.matmul(ps_us2[:], sel_s[:], u33[:, 512:1024],
                                 start=True, stop=True)
                us = vp.tile([2, F], dt, tag="us")
                nc.vector.tensor_copy(us[:, 0:512], ps_us[:])
                nc.vector.tensor_copy(us[:, 512:1024], ps_us2[:])
                # VS = [s; x] via partition-swapping SBUF->SBUF DMAs
                vs = vp.tile([2, F], dt, tag="vs")
                nc.sync.dma_start(out=vs[0:1, :], in_=us[1:2, :])
                nc.sync.dma_start(out=vs[1:2, :], in_=us[0:1, :])

                # X_sb[p, j, q] = X[f=128j+p, q]; X cols = [x, nb_0..nb_31]
                x_sb = xp.tile([128, 8, 33], dtb, tag="x")
                for j in range(8):
                    if _SKIP_TPOSE:
                        break
                    ps_t = pm.tile([128, 33], dt, tag="m")
                    nc.tensor.transpose(ps_t[:], u33[:, j * 128:(j + 1) * 128],
                                        idt_s[:33, :33])
                    nc.vector.tensor_copy(x_sb[:, j, :], ps_t[:])

                # fadj row-tiles -> sign; all 8 R_j kept live, then A@X.
                # NB: matmul start=True resets the WHOLE psum bank, so each
                # accumulation group needs its own bank (one tile per i).
                r_tiles = []
                for j in range(8):
                    r_j = rp.tile([128, F], dtb, tag="r")
                    for h in range(2):
                        if _SKIP_FADJ:
                            break
                        ps_f = pf.tile([128, 512], dt, tag="f")
                        nc.tensor.matmul(ps_f[:],
                                         us[:, j * 128:(j + 1) * 128],
                                         vs[:, h * 512:(h + 1) * 512],
                                         start=True, stop=True)
                        if not _SKIP_SIGN:
                            nc.scalar.sign(r_j[:, h * 512:(h + 1) * 512], ps_f[:])
                    r_tiles.append(r_j)

                # Yp[p, c, j] = (A@X)[f=128c+p, j]
                y_p = yp.tile([128, 8, 33], dt, tag="y")
                for i in range(8):
                    if _SKIP_NA:
                        break
                    ps_a = pa.tile([128, 33], dt, tag="acc")
                    for j in range(8):
                        nc.tensor.matmul(ps_a[:],
                                         r_tiles[j][:, i * 128:(i + 1) * 128],
                                         x_sb[:, j, :],
                                         start=(j == 0), stop=(j == 7))
                    nc.vector.tensor_copy(y_p[:, i, :], ps_a[:])

                ps_z = pm.tile([64, 33], dt, tag="m")
                for c in range(8):
                    nc.tensor.matmul(ps_z[:], w1t_s[:, c, :], y_p[:, c, :],
                                     start=(c == 0), stop=(c == 7))
                nc.vector.tensor_copy(z_t[:, n, :], ps_z[:])

            # ======== BN1 for neighbors (per-node stats) ========
            sq = wk.tile([64, NPC, 33], dt, tag="sq")
            nc.scalar.square(sq[:], z_t[:])
            ps_s = pm.tile([4, NPC, 33], dt, tag="m")
            ps_q = pm.tile([4, NPC, 33], dt, tag="m")
            nc.tensor.matmul(ps_s[:], g4_s[:],
                             z_t[:].rearrange("p n j -> p (n j)"),
                             start=True, stop=True)
            nc.tensor.matmul(ps_q[:], g4_s[:],
                             sq[:].rearrange("p n j -> p (n j)"),
                             start=True, stop=True)
            s_nb = wk.tile([4, NPC], dt, tag="snb")
            q_nb = wk.tile([4, NPC], dt, tag="qnb")
            nc.vector.tensor_reduce(s_nb[:], ps_s[:, :, 1:33], axis=AX.X, op=OP.add)
            nc.vector.tensor_reduce(q_nb[:], ps_q[:, :, 1:33], axis=AX.X, op=OP.add)
            m_nb = wk.tile([4, NPC], dt, tag="mnb")
            nc.vector.tensor_scalar_mul(m_nb[:], s_nb[:], 1.0 / 512)
            v_nb = wk.tile([4, NPC], dt, tag="vnb")
            nc.vector.tensor_scalar(v_nb[:], q_nb[:], 1.0 / 512, BN_EPS,
                                    OP.mult, OP.add)
            m2_nb = wk.tile([4, NPC], dt, tag="m2nb")
            nc.vector.tensor_mul(m2_nb[:], m_nb[:], m_nb[:])
            nc.vector.tensor_sub(v_nb[:], v_nb[:], m2_nb[:])
            nc.scalar.sqrt(v_nb[:], v_nb[:])
            is_nb = wk.tile([4, NPC], dt, tag="isnb")
            nc.vector.reciprocal(is_nb[:], v_nb[:])
            # broadcast c -> o=(c,f): MB[:, 0:8]=mean, [:, 8:16]=istd
            mb_in = wk.tile([4, 16], dt, tag="mbin")
            nc.vector.tensor_copy(mb_in[:, 0:NPC], m_nb[:])
            nc.vector.tensor_copy(mb_in[:, NPC:16], is_nb[:])
            ps_mb = pm.tile([64, 16], dt, tag="m")
            nc.tensor.matmul(ps_mb[:], bc4_s[:], mb_in[:], start=True, stop=True)
            mb = wk.tile([64, 16], dt, tag="mb")
            nc.vector.tensor_copy(mb[:], ps_mb[:])

            nb1 = wk.tile([64, NPC, K], dt, tag="nb1")
            for n in range(NPC):
                nc.vector.tensor_scalar(nb1[:, n, :], z_t[:, n, 1:33],
                                        mb[:, n:n + 1], mb[:, NPC + n:NPC + n + 1],
                                        OP.subtract, OP.mult)
            nc.vector.tensor_scalar(nb1[:], nb1[:], bnw1_s[:], bnb1_s[:],
                                    OP.mult, OP.add)
            ab1 = wk.tile([64, NPC, K], dt, tag="ab1")
            nc.scalar.activation(ab1[:], nb1[:],
                                 mybir.ActivationFunctionType.Abs)
            nc.vector.tensor_scalar_add(ab1[:], ab1[:], 1.0)
            nc.vector.reciprocal(ab1[:], ab1[:])
            nc.vector.tensor_mul(nb1[:], nb1[:], ab1[:])
            s2_loc = wk.tile([64, NPC], dt, tag="s2loc")
            nc.vector.tensor_reduce(s2_loc[:], nb1[:], axis=AX.X, op=OP.add)

            # ======== AllGather [x1_pre | S2] ========
            gl = wk.tile([64, 16], dt, tag="gl")
            nc.vector.tensor_copy(gl[:, 0:NPC], z_t[:, :, 0])
            nc.vector.tensor_copy(gl[:, NPC:16], s2_loc[:])
            gb = dp.tile([64, 16], dt)
            nc.sync.dma_start(out=gb[:], in_=gl[:])
            nc.gpsimd.collective_compute(
                "AllGather", OP.bypass,
                ins=[gb[:].opt()],
                outs=[gshared[:].opt()],
                replica_groups=[list(range(N_CORES))],
            )
            x1g = wk.tile([64, 64], dt, tag="x1g")
            nc.sync.dma_start(
                out=x1g[:].rearrange("p (r n) -> p r n", r=N_CORES),
                in_=gshared.ap().rearrange("r o c -> o r c")[:, :, 0:NPC])
            s2g = wk.tile([64, 64], dt, tag="s2g")
            nc.sync.dma_start(
                out=s2g[:].rearrange("p (r n) -> p r n", r=N_CORES),
                in_=gshared.ap().rearrange("r o c -> o r c")[:, :, NPC:16])

            # ======== BN1 for x (global stats) ========
            sqx = wk.tile([64, 64], dt, tag="sqx")
            nc.scalar.square(sqx[:], x1g[:])
            ps_sx = pm.tile([4, 64], dt, tag="m")
            ps_qx = pm.tile([4, 64], dt, tag="m")
            nc.tensor.matmul(ps_sx[:], g4_s[:], x1g[:], start=True, stop=True)
            nc.tensor.matmul(ps_qx[:], g4_s[:], sqx[:], start=True, stop=True)
            s_x = wk.tile([4, 1], dt, tag="sx")
            q_x = wk.tile([4, 1], dt, tag="qx")
            nc.vector.tensor_reduce(s_x[:], ps_sx[:], axis=AX.X, op=OP.add)
            nc.vector.tensor_reduce(q_x[:], ps_qx[:], axis=AX.X, op=OP.add)
            m_x = wk.tile([4, 1], dt, tag="mx")
            nc.vector.tensor_scalar_mul(m_x[:], s_x[:], 1.0 / 1024)
            v_x = wk.tile([4, 1], dt, tag="vx")
            nc.vector.tensor_scalar(v_x[:], q_x[:], 1.0 / 1024, BN_EPS,
                                    OP.mult, OP.add)
            m2_x = wk.tile([4, 1], dt, tag="m2x")
            nc.vector.tensor_mul(m2_x[:], m_x[:], m_x[:])
            nc.vector.tensor_sub(v_x[:], v_x[:], m2_x[:])
            nc.scalar.sqrt(v_x[:], v_x[:])
            is_x = wk.tile([4, 1], dt, tag="isx")
            nc.vector.reciprocal(is_x[:], v_x[:])
            mbx_in = wk.tile([4, 2], dt, tag="mbxin")
            nc.vector.tensor_copy(mbx_in[:, 0:1], m_x[:])
            nc.vector.tensor_copy(mbx_in[:, 1:2], is_x[:])
            ps_mbx = pm.tile([64, 2], dt, tag="m")
            nc.tensor.matmul(ps_mbx[:], bc4_s[:], mbx_in[:], start=True, stop=True)
            mbx = wk.tile([64, 2], dt, tag="mbx")
            nc.vector.tensor_copy(mbx[:], ps_mbx[:])

            x1bn = wk.tile([64, 64], dt, tag="x1bn")
            nc.vector.tensor_scalar(x1bn[:], x1g[:], mbx[:, 0:1], mbx[:, 1:2],
                                    OP.subtract, OP.mult)
            nc.vector.tensor_scalar(x1bn[:], x1bn[:], bnw1_s[:], bnb1_s[:],
                                    OP.mult, OP.add)
            abx = wk.tile([64, 64], dt, tag="abx")
            nc.scalar.activation(abx[:], x1bn[:],
                                 mybir.ActivationFunctionType.Abs)
            nc.vector.tensor_scalar_add(abx[:], abx[:], 1.0)
            nc.vector.reciprocal(abx[:], abx[:])
            nc.vector.tensor_mul(x1bn[:], x1bn[:], abx[:])

            # ======== layer 2 (all 64 nodes, redundant per core) ========
            ps_t1 = pm.tile([64, 64], dt, tag="m")
            nc.tensor.transpose(ps_t1[:], x1bn[:], idt_s[:64, :64])
            x1n = wk.tile([64, 64], dt, tag="x1n")
            nc.vector.tensor_copy(x1n[:], ps_t1[:])
            ps_t2 = pm.tile([64, 64], dt, tag="m")
            nc.tensor.transpose(ps_t2[:], s2g[:], idt_s[:64, :64])
            s2n = wk.tile([64, 64], dt, tag="s2n")
            nc.vector.tensor_copy(s2n[:], ps_t2[:])

            sh = [64, 4, 16, 16]
            x1_ca = x1n[:].rearrange("p (c a) -> p c a", c=4).unsqueeze(3).broadcast_to(sh)
            x1_cb = x1n[:].rearrange("p (c b) -> p c b", c=4).unsqueeze(2).broadcast_to(sh)
            s2_ca = s2n[:].rearrange("p (c a) -> p c a", c=4).unsqueeze(3).broadcast_to(sh)
            s2_cb = s2n[:].rearrange("p (c b) -> p c b", c=4).unsqueeze(2).broadcast_to(sh)

            f1 = wk.tile(sh, dt, tag="f1")
            f2 = wk.tile(sh, dt, tag="f2")
            nc.vector.tensor_mul(f1[:], x1_ca, s2_cb)
            nc.vector.tensor_mul(f2[:], x1_cb, s2_ca)
            nc.vector.tensor_add(f1[:], f1[:], f2[:])
            sg2 = wk.tile(sh, dt, tag="sg2")
            nc.scalar.sign(sg2[:], f1[:])
            a3 = wk.tile(sh, dt, tag="a3")
            nc.scalar.activation(a3[:], f1[:],
                                 mybir.ActivationFunctionType.Abs)
            nc.vector.tensor_scalar_max(a3[:], a3[:], 1e-8)
            nc.scalar.sqrt(a3[:], a3[:])
            sr = wk.tile(sh, dt, tag="sr")
            nc.vector.tensor_mul(sr[:], sg2[:], a3[:])
            d01 = wk.tile([64, 16, 16], dt, tag="d01")
            d23 = wk.tile([64, 16, 16], dt, tag="d23")
            nc.vector.tensor_add(d01[:], a3[:, 0], a3[:, 1])
            nc.vector.tensor_add(d23[:], a3[:, 2], a3[:, 3])
            nc.vector.tensor_add(d01[:], d01[:], d23[:])
            nc.vector.tensor_scalar_add(d01[:], d01[:], 1e-7)
            nc.vector.reciprocal(d01[:], d01[:])
            adj2 = wk.tile(sh, dt, tag="adj2")
            rd_b = d01[:].unsqueeze(1).broadcast_to(sh)
            nc.vector.tensor_mul(adj2[:], sr[:], rd_b)
            p2 = wk.tile(sh, dt, tag="p2")
            nc.vector.tensor_mul(p2[:], adj2[:], x1_cb)
            xa2 = wk.tile([64, 4, 16], dt, tag="xa2")
            nc.vector.tensor_reduce(xa2[:], p2[:], axis=AX.X, op=OP.add)
            ps_t3 = pm.tile([64, 64], dt, tag="m")
            nc.tensor.transpose(ps_t3[:], xa2[:].rearrange("p c a -> p (c a)"),
                                idt_s[:64, :64])
            xa2t = wk.tile([64, 64], dt, tag="xa2t")
            nc.vector.tensor_copy(xa2t[:], ps_t3[:])

            ps_x2 = pm.tile([32, 64], dt, tag="m")
            nc.tensor.matmul(ps_x2[:], w2t_s[:], xa2t[:], start=True, stop=True)
            x2 = wk.tile([32, 64], dt, tag="x2")
            nc.vector.tensor_copy(x2[:], ps_x2[:])

            # BN2 (stats over nodes) + softsign
            sq2 = wk.tile([32, 64], dt, tag="sq2")
            nc.scalar.square(sq2[:], x2[:])
            s_2 = wk.tile([32, 1], dt, tag="s2s")
            q_2 = wk.tile([32, 1], dt, tag="q2s")
            nc.vector.tensor_reduce(s_2[:], x2[:], axis=AX.X, op=OP.add)
            nc.vector.tensor_reduce(q_2[:], sq2[:], axis=AX.X, op=OP.add)
            m_2 = wk.tile([32, 1], dt, tag="m2s")
            nc.vector.tensor_scalar_mul(m_2[:], s_2[:], 1.0 / 64)
            v_2 = wk.tile([32, 1], dt, tag="v2s")
            nc.vector.tensor_scalar(v_2[:], q_2[:], 1.0 / 64, BN_EPS,
                                    OP.mult, OP.add)
            m22 = wk.tile([32, 1], dt, tag="m22s")
            nc.vector.tensor_mul(m22[:], m_2[:], m_2[:])
            nc.vector.tensor_sub(v_2[:], v_2[:], m22[:])
            nc.scalar.sqrt(v_2[:], v_2[:])
            is_2 = wk.tile([32, 1], dt, tag="is2s")
            nc.vector.reciprocal(is_2[:], v_2[:])
            nc.vector.tensor_scalar(x2[:], x2[:], m_2[:], is_2[:],
                                    OP.subtract, OP.mult)
            nc.vector.tensor_scalar(x2[:], x2[:], bnw2_s[:], bnb2_s[:],
                                    OP.mult, OP.add)
            ab2 = wk.tile([32, 64], dt, tag="ab2")
            nc.scalar.activation(ab2[:], x2[:],
                                 mybir.ActivationFunctionType.Abs)
            nc.vector.tensor_scalar_add(ab2[:], ab2[:], 1.0)
            nc.vector.reciprocal(ab2[:], ab2[:])
            nc.vector.tensor_mul(x2[:], x2[:], ab2[:])

            # linear head: [X2bn; ones]^T @ [lin_w.T; lin_b]
            l33 = wk.tile([33, 64], dt, tag="l33")
            nc.vector.tensor_copy(l33[0:32, :], x2[:])
            nc.vector.memset(l33[32:33, :], 1.0)
            ps_o = pm.tile([64, 10], dt, tag="m")
            nc.tensor.matmul(ps_o[:], l33[:], linw_s[:], start=True, stop=True)
            o_t = wk.tile([64, 10], dt, tag="ot")
            nc.vector.tensor_copy(o_t[:], ps_o[:])
            nc.sync.dma_start(out=out_d[:], in_=o_t[:])

            if _DEBUG:
                for nm, tl in [("dbg_z", z_t), ("dbg_nb1", nb1),
                               ("dbg_s2loc", s2_loc), ("dbg_x1g", x1g),
                               ("dbg_s2g", s2g), ("dbg_x1bn", x1bn),
                               ("dbg_f1", f1), ("dbg_a3", a3),
                               ("dbg_adj2", adj2), ("dbg_xa2", xa2),
                               ("dbg_x2", x2), ("dbg_us", us),
                               ("dbg_vs", vs), ("dbg_xsb", x_sb),
                               ("dbg_yp", y_p)]:
                    d = nc.dram_tensor(nm, list(tl.shape), dt,
                                       kind="ExternalOutput")
                    nc.sync.dma_start(out=d[:], in_=tl[:])

    nc.compile()
    return nc


def _in_maps(x, neighbor, W1, W2, bn1_w, bn1_b, bn2_w, bn2_b, lin_w, lin_b):
    f32 = np.float32
    x = np.ascontiguousarray(x, f32).reshape(64, F)
    nb = np.ascontiguousarray(neighbor, f32).reshape(64, K, F)
    w1f = np.ascontiguousarray(W1, f32).reshape(64, F)
    w1t = np.ascontiguousarray(w1f.T.reshape(8, 128, 64))
    sel = np.zeros((33, 2), f32)
    sel[0, 0] = 1.0
    sel[1:, 1] = 1.0
    idt = np.eye(128, dtype=f32)
    g4 = np.zeros((64, 4), f32)
    for c in range(4):
        g4[c * 16:(c + 1) * 16, c] = 1.0
    bc4 = np.ascontiguousarray(g4.T)
    bnw1v = np.repeat(np.asarray(bn1_w, f32), 16).reshape(64, 1)
    bnb1v = np.repeat(np.asarray(bn1_b, f32), 16).reshape(64, 1)
    w2t = np.ascontiguousarray(np.asarray(W2, f32).reshape(32, 64).T)
    bnw2v = np.asarray(bn2_w, f32).reshape(32, 1)
    bnb2v = np.asarray(bn2_b, f32).reshape(32, 1)
    linw = np.concatenate([np.asarray(lin_w, f32).T,
                           np.asarray(lin_b, f32).reshape(1, 10)], axis=0)
    maps = []
    for r in range(N_CORES):
        maps.append({
            "xs": np.ascontiguousarray(x[r * NPC:(r + 1) * NPC]),
            "nbs": np.ascontiguousarray(nb[r * NPC:(r + 1) * NPC]),
            "w1t": w1t, "sel": sel, "idt": idt, "g4": g4, "bc4": bc4,
            "bnw1": bnw1v, "bnb1": bnb1v, "w2t": w2t,
            "bnw2": bnw2v, "bnb2": bnb2v, "linw": linw,
        })
    return maps


def kernel(**inputs) -> np.ndarray:
    from concourse.bass_utils import run_bass_kernel_spmd
    if "nc" not in _CACHE:
        _CACHE["nc"] = _build()
    nc = _CACHE["nc"]
    maps = _in_maps(**inputs)
    res = run_bass_kernel_spmd(nc, maps, list(range(N_CORES)))
    return np.ascontiguousarray(res.results[0]["out"])



# revision 6
# speedup vs baseline: 1.3657x; 1.3657x over previous
"""Trainium2 Bass kernel for nn_LGL GNN message passing (N=64, K=32, F=1024).

Data-parallel over nodes: 8 nodes/core on 8 NeuronCores. Layer-1 adjacency
uses sign(fadj) (row-normalization for c=1 is sign() to <=1e-3). fadj rank-2
products run on the PE in f32r (measured ~3e-4 matmul error, well under the
sign-flip tolerance). The sign conversion psum->sbuf alternates between the
Activation engine (Sign, +-1, W1) and DVE (is_gt - 0.5, +-0.5, 2*W1) per
node to split the elementwise load. s = sum_k neighbor is computed on the
idle GPSIMD engine via partition_all_reduce. Cross-core traffic is two tiny
AllGathers of BN partial sums (BN1-x and BN2 batch stats); each core
computes layer 2 + head only for its own 8 nodes and the host concatenates
the per-core logits.
"""
import numpy as np

N_CORES = 8
NPC = 8          # nodes per core
F = 1024
K = 32
BN_EPS = 1e-5

_CACHE = {}
_DEBUG = False


def _build():
    import concourse.bacc as bacc
    import concourse.mybir as mybir
    import concourse.bass_isa as bass_isa
    import concourse.tile as tile

    dt = mybir.dt.float32
    dtr = mybir.dt.float32r
    dtb = mybir.dt.bfloat16
    AX = mybir.AxisListType
    OP = mybir.AluOpType
    AF = mybir.ActivationFunctionType
    RED = bass_isa.ReduceOp

    nc = bacc.Bacc("TRN2", target_bir_lowering=False, debug=False)

    xs = nc.dram_tensor("xs", [NPC, F], dtr, kind="ExternalInput")
    nbs = nc.dram_tensor("nbs", [NPC, K, F], dtr, kind="ExternalInput")
    idt64 = nc.dram_tensor("idt64", [64, 64], dt, kind="ExternalInput")
    w1tb = nc.dram_tensor("w1tb", [8, 128, 64], dtb, kind="ExternalInput")
    w1tb2 = nc.dram_tensor("w1tb2", [8, 128, 64], dtb, kind="ExternalInput")
    g4 = nc.dram_tensor("g4", [64, 4], dt, kind="ExternalInput")
    bc4 = nc.dram_tensor("bc4", [4, 64], dt, kind="ExternalInput")
    bnw1 = nc.dram_tensor("bnw1", [64, 1], dt, kind="ExternalInput")
    bnb1 = nc.dram_tensor("bnb1", [64, 1], dt, kind="ExternalInput")
    w2t = nc.dram_tensor("w2t", [64, 32], dt, kind="ExternalInput")
    bnw2 = nc.dram_tensor("bnw2", [32, 1], dt, kind="ExternalInput")
    bnb2 = nc.dram_tensor("bnb2", [32, 1], dt, kind="ExternalInput")
    linw = nc.dram_tensor("linw", [33, 10], dt, kind="ExternalInput")
    out_d = nc.dram_tensor("out", [NPC, 10], dt, kind="ExternalOutput")
    gsh1 = nc.dram_tensor("gsh1", [N_CORES, 4, 2], dt, addr_space="Shared")
    gsh2 = nc.dram_tensor("gsh2", [N_CORES, 32, 2], dt, addr_space="Shared")

    with tile.TileContext(nc) as tc:
        with (
            tc.tile_pool(name="wpool", bufs=1) as wp,
            tc.tile_pool(name="upool", bufs=2) as up,
            tc.tile_pool(name="spool", bufs=2) as sp,
            tc.tile_pool(name="uvpool", bufs=2) as uv,
            tc.tile_pool(name="xpool", bufs=2) as xp,
            tc.tile_pool(name="rpool", bufs=12) as rp,
            tc.tile_pool(name="ypool", bufs=2) as yp,
            tc.tile_pool(name="work", bufs=1) as wk,
            tc.tile_pool(name="psum", bufs=1, space="PSUM") as pp,
            tc.tile_pool(name="dram", bufs=1, space="DRAM") as dp,
        ):
            # ---- load weights / constants ----
            idt64_s = wp.tile([64, 64], dt)
            nc.sync.dma_start(out=idt64_s[:], in_=idt64[:])
            w1tb_s = wp.tile([128, 8, 64], dtb)
            nc.sync.dma_start(out=w1tb_s[:], in_=w1tb.ap().rearrange("c p o -> p c o"))
            w1tb2_s = wp.tile([128, 8, 64], dtb)
            nc.sync.dma_start(out=w1tb2_s[:], in_=w1tb2.ap().rearrange("c p o -> p c o"))
            g4_s = wp.tile([64, 4], dt)
            nc.sync.dma_start(out=g4_s[:], in_=g4[:])
            bc4_s = wp.tile([4, 64], dt)
            nc.sync.dma_start(out=bc4_s[:], in_=bc4[:])
            bnw1_s = wp.tile([64, 1], dt)
            nc.sync.dma_start(out=bnw1_s[:], in_=bnw1[:])
            bnb1_s = wp.tile([64, 1], dt)
            nc.sync.dma_start(out=bnb1_s[:], in_=bnb1[:])
            w2t_s = wp.tile([64, 32], dt)
            nc.sync.dma_start(out=w2t_s[:], in_=w2t[:])
            bnw2_s = wp.tile([32, 1], dt)
            nc.sync.dma_start(out=bnw2_s[:], in_=bnw2[:])
            bnb2_s = wp.tile([32, 1], dt)
            nc.sync.dma_start(out=bnb2_s[:], in_=bnb2[:])
            linw_s = wp.tile([33, 10], dt)
            nc.sync.dma_start(out=linw_s[:], in_=linw[:])

            # Z[o, n, j]: layer-1 raw outputs; cols 0:32 = nb, col 32 = x
            z_t = wk.tile([64, NPC, 33], dt, tag="z")

            # ======== layer 1, per local node ========
            for n in range(NPC):
                act_node = (n % 2 == 0)   # sign engine for this node

                u33 = up.tile([33, F], dtr, tag="u33")
                nc.sync.dma_start(out=u33[0:32, :], in_=nbs[n, :, :])
                nc.sync.dma_start(out=u33[32:33, :], in_=xs[n:n + 1, :])

                s32 = sp.tile([32, F], dtr, tag="s32")
                nc.gpsimd.partition_all_reduce(s32[:], u33[0:32, :], 32, RED.add)

                us = uv.tile([2, F], dtr, tag="us")
                vs = uv.tile([2, F], dtr, tag="vs")
                nc.sync.dma_start(out=us[0:1, :], in_=u33[32:33, :])
                nc.sync.dma_start(out=us[1:2, :], in_=s32[0:1, :])
                nc.sync.dma_start(out=vs[0:1, :], in_=s32[1:2, :])
                nc.sync.dma_start(out=vs[1:2, :], in_=u33[32:33, :])

                # X_sb[p, j, q] = X[f=128j+p, q]; X cols = [nb_0..nb_31, x]
                ps_pack = pp.tile([128, 8, 33], dt, tag="pack", bufs=2)
                if act_node:
                    nc.vector.memset(ps_pack[:], 0.0)
                else:
                    nc.scalar.memzero(ps_pack[:])
                for j in range(8):
                    nc.tensor.matmul(ps_pack[:, j, :],
                                     u33[:, j * 128:(j + 1) * 128].bitcast(dt),
                                     idt64_s[:33, :33], is_transpose=True,
                                     start=False, stop=(j == 7),
                                     skip_group_check=True)
                x_sb = xp.tile([128, 8, 33], dtb, tag="x")
                if act_node:
                    nc.vector.tensor_copy(
                        x_sb[:].rearrange("p a b -> p (a b)"),
                        ps_pack[:].rearrange("p a b -> p (a b)"))
                else:
                    nc.scalar.copy(
                        x_sb[:].rearrange("p a b -> p (a b)"),
                        ps_pack[:].rearrange("p a b -> p (a b)"))

                # fadj chunks -> sign into r tiles
                r_tiles = []
                for j in range(8):
                    r_j = rp.tile([128, F], dtb, tag="r")
                    for h in range(2):
                        ps_f = pp.tile([128, 512], dt, tag="f", bufs=3)
                        nc.tensor.matmul(ps_f[:],
                                         us[:, j * 128:(j + 1) * 128],
                                         vs[:, h * 512:(h + 1) * 512],
                                         start=True, stop=True)
                        if act_node:
                            nc.scalar.sign(r_j[:, h * 512:(h + 1) * 512],
                                           ps_f[:])
                        else:
                            nc.vector.tensor_scalar(
                                r_j[:, h * 512:(h + 1) * 512], ps_f[:],
                                0.0, 0.5, op0=OP.is_gt, op1=OP.subtract)
                    r_tiles.append(r_j)

                # Y[p, i, q] = (A@X)[f=128i+p, q], packed accumulation
                ps_acc = pp.tile([128, 8, 33], dt, tag="acc", bufs=2)
                if act_node:
                    nc.vector.memset(ps_acc[:], 0.0)
                else:
                    nc.scalar.memzero(ps_acc[:])
                for i in range(8):
                    for j in range(8):
                        nc.tensor.matmul(ps_acc[:, i, :],
                                         r_tiles[j][:, i * 128:(i + 1) * 128],
                                         x_sb[:, j, :],
                                         start=False, stop=(j == 7),
                                         skip_group_check=True)
                y_sb = yp.tile([128, 8, 33], dtb, tag="y")
                if act_node:
                    nc.vector.tensor_copy(
                        y_sb[:].rearrange("p a b -> p (a b)"),
                        ps_acc[:].rearrange("p a b -> p (a b)"))
                else:
                    nc.scalar.copy(
                        y_sb[:].rearrange("p a b -> p (a b)"),
                        ps_acc[:].rearrange("p a b -> p (a b)"))

                ps_z = pp.tile([64, 33], dt, tag="zz", bufs=1)
                w1use = w1tb_s if act_node else w1tb2_s
                for c in range(8):
                    nc.tensor.matmul(ps_z[:], w1use[:, c, :], y_sb[:, c, :],
                                     start=(c == 0), stop=(c == 7))
                if act_node:
                    nc.vector.tensor_copy(z_t[:, n, :], ps_z[:])
                else:
                    nc.scalar.copy(z_t[:, n, :], ps_z[:])

            # ======== BN1 for neighbors (per-node stats over k,f) ========
            sq = wk.tile([64, NPC, 33], dt, tag="sq")
            nc.scalar.square(sq[:], z_t[:])
            ps_s = pp.tile([4, NPC, 33], dt, tag="zz", bufs=1)
            nc.tensor.matmul(ps_s[:], g4_s[:],
                             z_t[:].rearrange("p n j -> p (n j)"),
                             start=True, stop=True)
            ps_q = pp.tile([4, NPC, 33], dt, tag="pack", bufs=2)
            nc.tensor.matmul(ps_q[:], g4_s[:],
                             sq[:].rearrange("p n j -> p (n j)"),
                             start=True, stop=True)
            s_nb = wk.tile([4, NPC], dt, tag="snb")
            q_nb = wk.tile([4, NPC], dt, tag="qnb")
            nc.vector.tensor_reduce(s_nb[:], ps_s[:, :, 0:32], axis=AX.X, op=OP.add)
            nc.vector.tensor_reduce(q_nb[:], ps_q[:, :, 0:32], axis=AX.X, op=OP.add)
            m_nb = wk.tile([4, NPC], dt, tag="mnb")
            nc.vector.tensor_scalar_mul(m_nb[:], s_nb[:], 1.0 / 512)
            v_nb = wk.tile([4, NPC], dt, tag="vnb")
            nc.vector.tensor_scalar(v_nb[:], q_nb[:], 1.0 / 512, BN_EPS,
                                    op0=OP.mult, op1=OP.add)
            m2_nb = wk.tile([4, NPC], dt, tag="m2nb")
            nc.vector.tensor_mul(m2_nb[:], m_nb[:], m_nb[:])
            nc.vector.tensor_sub(v_nb[:], v_nb[:], m2_nb[:])
            nc.scalar.sqrt(v_nb[:], v_nb[:])
            is_nb = wk.tile([4, NPC], dt, tag="isnb")
            nc.vector.reciprocal(is_nb[:], v_nb[:])
            # broadcast c -> o=(c,f): MB[:, 0:8]=mean, [:, 8:16]=istd
            mb_in = wk.tile([4, 16], dt, tag="mbin")
            nc.vector.tensor_copy(mb_in[:, 0:NPC], m_nb[:])
            nc.vector.tensor_copy(mb_in[:, NPC:16], is_nb[:])
            ps_mb = pp.tile([64, 16], dt, tag="zz", bufs=1)
            nc.tensor.matmul(ps_mb[:], bc4_s[:], mb_in[:], start=True, stop=True)
            mb = wk.tile([64, 16], dt, tag="mb")
            nc.vector.tensor_copy(mb[:], ps_mb[:])

            nb1 = wk.tile([64, NPC, K], dt, tag="nb1")
            for n in range(NPC):
                nc.vector.tensor_scalar(nb1[:, n, :], z_t[:, n, 0:32],
                                        mb[:, n:n + 1], mb[:, NPC + n:NPC + n + 1],
                                        op0=OP.subtract, op1=OP.mult)
            nc.vector.tensor_scalar(nb1[:], nb1[:], bnw1_s[:], bnb1_s[:],
                                    op0=OP.mult, op1=OP.add)
            ab1 = wk.tile([64, NPC, K], dt, tag="ab1")
            nc.scalar.activation(ab1[:], nb1[:], AF.Abs)
            nc.gpsimd.tensor_scalar_add(ab1[:], ab1[:], 1.0)
            nc.vector.reciprocal(ab1[:], ab1[:])
            nc.vector.tensor_mul(nb1[:], nb1[:], ab1[:])
            s2_loc = wk.tile([64, NPC], dt, tag="s2loc")
            nc.vector.tensor_reduce(s2_loc[:], nb1[:], axis=AX.X, op=OP.add)

            # ======== BN1-x: allgather partial sums over cores ========
            zx = wk.tile([64, NPC], dt, tag="zx")
            nc.vector.tensor_copy(zx[:], z_t[:, :, 32])
            sqx = wk.tile([64, NPC], dt, tag="sqx")
            nc.scalar.square(sqx[:], zx[:])
            ps_sx = pp.tile([4, NPC], dt, tag="zz", bufs=1)
            nc.tensor.matmul(ps_sx[:], g4_s[:], zx[:], start=True, stop=True)
            ps_qx = pp.tile([4, NPC], dt, tag="pack", bufs=2)
            nc.tensor.matmul(ps_qx[:], g4_s[:], sqx[:], start=True, stop=True)
            gl1 = wk.tile([4, 2], dt, tag="gl1")
            nc.vector.tensor_reduce(gl1[:, 0:1], ps_sx[:], axis=AX.X, op=OP.add)
            nc.vector.tensor_reduce(gl1[:, 1:2], ps_qx[:], axis=AX.X, op=OP.add)
            gb1 = dp.tile([4, 2], dt)
            nc.sync.dma_start(out=gb1[:], in_=gl1[:])
            nc.gpsimd.collective_compute(
                "AllGather", OP.bypass,
                ins=[gb1[:].opt()],
                outs=[gsh1[:].opt()],
                replica_groups=[list(range(N_CORES))],
            )
            gx1 = wk.tile([4, 2, N_CORES], dt, tag="gx1")
            nc.sync.dma_start(
                out=gx1[:], in_=gsh1.ap().rearrange("r c k -> c k r"))
            sx4 = wk.tile([4, 2], dt, tag="sx4")
            nc.vector.tensor_reduce(sx4[:], gx1[:], axis=AX.X, op=OP.add)
            m_x = wk.tile([4, 1], dt, tag="mx")
            nc.vector.tensor_scalar_mul(m_x[:], sx4[:, 0:1], 1.0 / 1024)
            v_x = wk.tile([4, 1], dt, tag="vx")
            nc.vector.tensor_scalar(v_x[:], sx4[:, 1:2], 1.0 / 1024, BN_EPS,
                                    op0=OP.mult, op1=OP.add)
            m2_x = wk.tile([4, 1], dt, tag="m2x")
            nc.vector.tensor_mul(m2_x[:], m_x[:], m_x[:])
            nc.vector.tensor_sub(v_x[:], v_x[:], m2_x[:])
            nc.scalar.sqrt(v_x[:], v_x[:])
            is_x = wk.tile([4, 1], dt, tag="isx")
            nc.vector.reciprocal(is_x[:], v_x[:])
            mbx_in = wk.tile([4, 2], dt, tag="mbxin")
            nc.vector.tensor_copy(mbx_in[:, 0:1], m_x[:])
            nc.vector.tensor_copy(mbx_in[:, 1:2], is_x[:])
            ps_mbx = pp.tile([64, 2], dt, tag="zz", bufs=1)
            nc.tensor.matmul(ps_mbx[:], bc4_s[:], mbx_in[:], start=True, stop=True)
            mbx = wk.tile([64, 2], dt, tag="mbx")
            nc.vector.tensor_copy(mbx[:], ps_mbx[:])

            x1bn = wk.tile([64, NPC], dt, tag="x1bn")
            nc.vector.tensor_scalar(x1bn[:], zx[:], mbx[:, 0:1], mbx[:, 1:2],
                                    op0=OP.subtract, op1=OP.mult)
            nc.vector.tensor_scalar(x1bn[:], x1bn[:], bnw1_s[:], bnb1_s[:],
                                    op0=OP.mult, op1=OP.add)
            abx = wk.tile([64, NPC], dt, tag="abx")
            nc.scalar.activation(abx[:], x1bn[:], AF.Abs)
            nc.vector.tensor_scalar_add(abx[:], abx[:], 1.0)
            nc.vector.reciprocal(abx[:], abx[:])
            nc.vector.tensor_mul(x1bn[:], x1bn[:], abx[:])

            # ======== layer 2 (local 8 nodes only) ========
            ps_t1 = pp.tile([NPC, 64], dt, tag="f", bufs=3)
            nc.tensor.transpose(ps_t1[:], x1bn[:], idt64_s[:])
            x1n = wk.tile([NPC, 64], dt, tag="x1n")
            nc.vector.tensor_copy(x1n[:], ps_t1[:])
            ps_t2 = pp.tile([NPC, 64], dt, tag="f", bufs=3)
            nc.tensor.transpose(ps_t2[:], s2_loc[:], idt64_s[:])
            s2n = wk.tile([NPC, 64], dt, tag="s2n")
            nc.vector.tensor_copy(s2n[:], ps_t2[:])

            sh = [NPC, 4, 16, 16]
            x1_ca = x1n[:].rearrange("p (c a) -> p c a", c=4).unsqueeze(3).broadcast_to(sh)
            x1_cb = x1n[:].rearrange("p (c b) -> p c b", c=4).unsqueeze(2).broadcast_to(sh)
            s2_ca = s2n[:].rearrange("p (c a) -> p c a", c=4).unsqueeze(3).broadcast_to(sh)
            s2_cb = s2n[:].rearrange("p (c b) -> p c b", c=4).unsqueeze(2).broadcast_to(sh)

            f1 = wk.tile(sh, dt, tag="f1")
            f2 = wk.tile(sh, dt, tag="f2")
            nc.vector.tensor_mul(f1[:], x1_ca, s2_cb)
            nc.gpsimd.tensor_tensor(f2[:], x1_cb, s2_ca, op=OP.mult)
            nc.vector.tensor_add(f1[:], f1[:], f2[:])
            sg2 = wk.tile(sh, dt, tag="sg2")
            nc.scalar.sign(sg2[:], f1[:])
            a3 = wk.tile(sh, dt, tag="a3")
            nc.scalar.activation(a3[:], f1[:], AF.Abs)
            b8c = wk.tile([NPC, 1], dt, tag="b8c")
            nc.vector.memset(b8c[:], 1e-8)
            nc.scalar.activation(a3[:], a3[:], AF.Sqrt, bias=b8c[:])
            sr = wk.tile(sh, dt, tag="sr")
            nc.vector.tensor_mul(sr[:], sg2[:], a3[:])
            d01 = wk.tile([NPC, 16, 16], dt, tag="d01")
            d23 = wk.tile([NPC, 16, 16], dt, tag="d23")
            nc.vector.tensor_add(d01[:], a3[:, 0], a3[:, 1])
            nc.gpsimd.tensor_tensor(d23[:], a3[:, 2], a3[:, 3], op=OP.add)
            nc.vector.tensor_add(d01[:], d01[:], d23[:])
            nc.vector.tensor_scalar_add(d01[:], d01[:], 1e-7)
            nc.vector.reciprocal(d01[:], d01[:])
            adj2 = wk.tile(sh, dt, tag="adj2")
            rd_b = d01[:].unsqueeze(1).broadcast_to(sh)
            nc.vector.tensor_mul(adj2[:], sr[:], rd_b)
            p2 = wk.tile(sh, dt, tag="p2")
            nc.vector.tensor_mul(p2[:], adj2[:], x1_cb)
            xa2 = wk.tile([NPC, 4, 16], dt, tag="xa2")
            nc.vector.tensor_reduce(xa2[:], p2[:], axis=AX.X, op=OP.add)
            ps_t3 = pp.tile([64, NPC], dt, tag="f", bufs=3)
            nc.tensor.transpose(ps_t3[:], xa2[:].rearrange("p c a -> p (c a)"),
                                idt64_s[:NPC, :NPC])
            xa2t = wk.tile([64, NPC], dt, tag="xa2t")
            nc.vector.tensor_copy(xa2t[:], ps_t3[:])

            ps_x2 = pp.tile([32, NPC], dt, tag="zz", bufs=1)
            nc.tensor.matmul(ps_x2[:], w2t_s[:], xa2t[:], start=True, stop=True)
            x2 = wk.tile([32, NPC], dt, tag="x2")
            nc.vector.tensor_copy(x2[:], ps_x2[:])

            # ======== BN2: allgather partial sums, then softsign ========
            sq2 = wk.tile([32, NPC], dt, tag="sq2")
            nc.scalar.square(sq2[:], x2[:])
            gl2 = wk.tile([32, 2], dt, tag="gl2")
            nc.vector.tensor_reduce(gl2[:, 0:1], x2[:], axis=AX.X, op=OP.add)
            nc.vector.tensor_reduce(gl2[:, 1:2], sq2[:], axis=AX.X, op=OP.add)
            gb2 = dp.tile([32, 2], dt)
            nc.sync.dma_start(out=gb2[:], in_=gl2[:])
            nc.gpsimd.collective_compute(
                "AllGather", OP.bypass,
                ins=[gb2[:].opt()],
                outs=[gsh2[:].opt()],
                replica_groups=[list(range(N_CORES))],
            )
            gx2 = wk.tile([32, 2, N_CORES], dt, tag="gx2")
            nc.sync.dma_start(
                out=gx2[:], in_=gsh2.ap().rearrange("r c k -> c k r"))
            sx32 = wk.tile([32, 2], dt, tag="sx32")
            nc.vector.tensor_reduce(sx32[:], gx2[:], axis=AX.X, op=OP.add)
            m_2 = wk.tile([32, 1], dt, tag="m2s")
            nc.vector.tensor_scalar_mul(m_2[:], sx32[:, 0:1], 1.0 / 64)
            v_2 = wk.tile([32, 1], dt, tag="v2s")
            nc.vector.tensor_scalar(v_2[:], sx32[:, 1:2], 1.0 / 64, BN_EPS,
                                    op0=OP.mult, op1=OP.add)
            m22 = wk.tile([32, 1], dt, tag="m22s")
            nc.vector.tensor_mul(m22[:], m_2[:], m_2[:])
            nc.vector.tensor_sub(v_2[:], v_2[:], m22[:])
            nc.scalar.sqrt(v_2[:], v_2[:])
            is_2 = wk.tile([32, 1], dt, tag="is2s")
            nc.vector.reciprocal(is_2[:], v_2[:])
            nc.vector.tensor_scalar(x2[:], x2[:], m_2[:], is_2[:],
                                    op0=OP.subtract, op1=OP.mult)
            nc.vector.tensor_scalar(x2[:], x2[:], bnw2_s[:], bnb2_s[:],
                                    op0=OP.mult, op1=OP.add)
            ab2 = wk.tile([32, NPC], dt, tag="ab2")
            nc.scalar.activation(ab2[:], x2[:], AF.Abs)
            nc.vector.tensor_scalar_add(ab2[:], ab2[:], 1.0)
            nc.vector.reciprocal(ab2[:], ab2[:])
            nc.vector.tensor_mul(x2[:], x2[:], ab2[:])

            # linear head: [X2bn; ones]^T @ [lin_w.T; lin_b]
            l33 = wk.tile([33, NPC], dt, tag="l33")
            nc.vector.tensor_copy(l33[0:32, :], x2[:])
            nc.vector.memset(l33[32:33, :], 1.0)
            ps_o = pp.tile([NPC, 10], dt, tag="zz", bufs=1)
            nc.tensor.matmul(ps_o[:], l33[:], linw_s[:], start=True, stop=True)
            o_t = wk.tile([NPC, 10], dt, tag="ot")
            nc.vector.tensor_copy(o_t[:], ps_o[:])
            nc.sync.dma_start(out=out_d[:], in_=o_t[:])

            if _DEBUG:
                for nm, tl in [("dbg_z", z_t), ("dbg_nb1", nb1),
                               ("dbg_s2loc", s2_loc), ("dbg_x1bn", x1bn),
                               ("dbg_xa2", xa2), ("dbg_x2", x2),
                               ("dbg_gx1", gx1)]:
                    d = nc.dram_tensor(nm, list(tl.shape), dt,
                                       kind="ExternalOutput")
                    nc.sync.dma_start(out=d[:], in_=tl[:])

    nc.compile()
    return nc


def _in_maps(x, neighbor, W1, W2, bn1_w, bn1_b, bn2_w, bn2_b, lin_w, lin_b):
    f32 = np.float32
    bf16 = np.dtype('bfloat16') if hasattr(np, 'bfloat16') else None
    import ml_dtypes
    bf16 = ml_dtypes.bfloat16
    x = np.ascontiguousarray(x, f32).reshape(64, F)
    nb = np.ascontiguousarray(neighbor, f32).reshape(64, K, F)
    w1f = np.ascontiguousarray(W1, f32).reshape(64, F)
    w1t = np.ascontiguousarray(w1f.T.reshape(8, 128, 64))
    w1tb = w1t.astype(bf16)
    w1tb2 = (2.0 * w1t).astype(bf16)
    idt64 = np.eye(64, dtype=f32)
    g4 = np.zeros((64, 4), f32)
    for c in range(4):
        g4[c * 16:(c + 1) * 16, c] = 1.0
    bc4 = np.ascontiguousarray(g4.T)
    bnw1v = np.repeat(np.asarray(bn1_w, f32), 16).reshape(64, 1)
    bnb1v = np.repeat(np.asarray(bn1_b, f32), 16).reshape(64, 1)
    w2t = np.ascontiguousarray(np.asarray(W2, f32).reshape(32, 64).T)
    bnw2v = np.asarray(bn2_w, f32).reshape(32, 1)
    bnb2v = np.asarray(bn2_b, f32).reshape(32, 1)
    linw = np.concatenate([np.asarray(lin_w, f32).T,
                           np.asarray(lin_b, f32).reshape(1, 10)], axis=0)
    maps = []
    for r in range(N_CORES):
        maps.append({
            "xs": np.ascontiguousarray(x[r * NPC:(r + 1) * NPC]),
            "nbs": np.ascontiguousarray(nb[r * NPC:(r + 1) * NPC]),
            "idt64": idt64,
            "w1tb": w1tb, "w1tb2": w1tb2,
            "g4": g4, "bc4": bc4,
            "bnw1": bnw1v, "bnb1": bnb1v, "w2t": w2t,
            "bnw2": bnw2v, "bnb2": bnb2v, "linw": linw,
        })
    return maps


def kernel(**inputs) -> np.ndarray:
    from concourse.bass_utils import run_bass_kernel_spmd
    if "nc" not in _CACHE:
        _CACHE["nc"] = _build()
    nc = _CACHE["nc"]
    maps = _in_maps(**inputs)
    res = run_bass_kernel_spmd(nc, maps, list(range(N_CORES)))
    out = np.concatenate(
        [np.asarray(res.results[r]["out"]) for r in range(N_CORES)], axis=0)
    return np.ascontiguousarray(out, np.float32)
